# revision 44
# baseline (speedup 1.0000x reference)
"""Bahdanau additive attention kernel for Trainium2 (8 NeuronCores).

Sharding: 8 cores = 4 batches x 2 halves of the 32 target positions.
Each core handles one batch element b and 16 t's with the full source
length S=1024, so no cross-device softmax reduction is needed.

Per-core device algorithm (d = feature index in [0, 2H), on partitions):
  statesT via PE transposes (fp16)       (2H, S)
  sfT  = Ws^T @ statesT  (fp16 inputs, fp32 accumulate)
  for t, d-chunk: tanh(sfT + qfbT[:, t]) via ACT bias (fused add+tanh)
  align[t] = sum_d v_d * tanh            (PSUM-accumulated matmuls, M=1)
  maskterm added during the PSUM->SBUF stage copy
  softmax over s batched across the 16 t-rows (ACT exp with accum_out)
  context / contextT / attention_hidden via small f32r matmuls

Precision: the sf path uses fp16 inputs (the PE truncates fp32 to ~FP22
anyway and fp16 halves the startup-critical DMA); everything else uses
float32r (fp32 bytes, FP22 multiply, fp32 accumulate). Total relative
error vs the fp32 reference is ~2.5e-4.

qfbT (= (query @ Wq + bq)^T) and maskterm (= (mask-1)*1e9) are computed
on the host: together they are <0.2% of the FLOPs but sit on the device
critical path / enable the fused ACT-bias tanh.
"""

import numpy as np

import concourse.bass as bass
import concourse.tile as tile
from concourse import bacc, mybir
from concourse.bass_utils import run_bass_kernel_spmd
from concourse.masks import make_identity

P = 128
B, T, S = 4, 32, 1024
H = 512
D2 = 2 * H          # 1024
D3 = 3 * H          # 1536
TC = 16             # t's per core
NB = S // P         # 8 source chunks
KD2 = D2 // P       # 8
KH = H // P         # 4
KD3 = D3 // P       # 12
NCORES = 8

f32 = mybir.dt.float32
f32r = mybir.dt.float32r
f16 = mybir.dt.float16
AF = mybir.ActivationFunctionType


def _build():
    nc = bacc.Bacc("TRN2", target_bir_lowering=False)

    # smallpack = host-packed [qfbT | vT | qT] in SBUF partition layout:
    # one DMA instead of three (each small DMA costs ~0.6us of DMA-engine
    # overhead on the startup critical path).
    pack_d = nc.dram_tensor("smallpack", [P, 200], f32, kind="ExternalInput")
    states_d = nc.dram_tensor("states", [S, D2], f32, kind="ExternalInput")
    statesT_h_d = nc.dram_tensor("statesT_h", [D2, S], f16, kind="ExternalInput")
    maskterm_d = nc.dram_tensor("maskterm", [1, S], f32, kind="ExternalInput")
    # Ws in dout-major packing: WsP[dout, din*128+p, j] = Ws[din*128+p,
    # dout*128+j], so the first sfT group's weights arrive in the first
    # 256KB DMA instead of after the full 2MB.
    Ws_h_d = nc.dram_tensor("Ws_h", [KD2, P, KD2, P], f16, kind="ExternalInput")
    Wc_d = nc.dram_tensor("Wc", [D3, H], f32, kind="ExternalInput")
    Wc1_h_d = nc.dram_tensor("Wc1_h", [D2, H], f16, kind="ExternalInput")
    bc_d = nc.dram_tensor("bc", [1, H], f32, kind="ExternalInput")

    ctx_d = nc.dram_tensor("out_ctx", [TC, D2], f32, kind="ExternalOutput")
    hid_d = nc.dram_tensor("out_hid", [TC, H], f32, kind="ExternalOutput")
    attT_d = nc.dram_tensor("out_attT", [S, TC], f32, kind="ExternalOutput")

    with tile.TileContext(nc) as tc:
        with (
            tc.tile_pool(name="singles", bufs=1) as singles,
            tc.tile_pool(name="tanhp", bufs=3) as tanhp,
            tc.tile_pool(name="stagep", bufs=2) as stagep,
            tc.tile_pool(name="ps_big", bufs=2, space="PSUM") as ps_big,
            tc.tile_pool(name="ps_al", bufs=6, space="PSUM") as ps_al,
        ):
            # ---- constants computed on-chip (no DMA cost) ----
            ones_f32 = singles.tile([1, TC], f32, tag="ones_f32")
            nc.vector.memset(ones_f32, 1.0)
            # Dummy activation to pull the ~1.3us ACT table load (tanh/exp
            # share one table set) off the first real tanh's critical path.
            actwarm = singles.tile([1, 1], f32, tag="actwarm")
            nc.scalar.activation(actwarm, ones_f32[:, 0:1], AF.Tanh)
            ones1 = singles.tile([1, 1], f32r, tag="ones1")
            nc.vector.tensor_copy(ones1, ones_f32[:, 0:1])
            onesT = singles.tile([1, TC], f32r, tag="onesT")
            nc.vector.tensor_copy(onesT, ones_f32)
            ident_f32 = singles.tile([P, P], f32, tag="ident_f32")
            make_identity(nc, ident_f32)
            ident = singles.tile([P, P], f32r, tag="ident")
            nc.vector.tensor_copy(ident, ident_f32)
            # Ramp the PE's HAM clock before the first real sfT matmuls land
            # (~6us in): a cold PE runs at half rate for its first ~3.4us.
            warm0 = ps_big.tile([1, P], f32, tag="mm512", padded_shape=[P, 512])
            for _ in range(30):
                nc.tensor.matmul(
                    warm0, ones1, ident[0:1, :],
                    start=True, stop=True, skip_group_check=True,
                )

            # ---- fp16 statesT (host-pre-transposed) + Ws loads: the
            # startup-critical path gating the sfT matmuls; DMA'd first ----
            statesT_sb = [singles.tile([P, S], f16, tag=f"stT{d}", name=f"stT{d}")
                          for d in range(KD2)]
            Ws_sb = [singles.tile([P, KD2, P], f16, tag=f"Ws{k}", name=f"Ws{k}")
                     for k in range(KD2)]
            for d in range(4):
                nc.sync.dma_start(
                    statesT_sb[d], statesT_h_d[d * P:(d + 1) * P, :])
            nc.sync.dma_start(Ws_sb[0], Ws_h_d[0])
            for d in range(4, KD2):
                nc.sync.dma_start(
                    statesT_sb[d], statesT_h_d[d * P:(d + 1) * P, :])

            # ---- small inputs (packed; needed from the first tanh on) ----
            pack_sb = singles.tile([P, 200], f32r, tag="pack")
            nc.sync.dma_start(pack_sb, pack_d[:].bitcast(f32r))
            for k in range(1, KD2):
                nc.sync.dma_start(Ws_sb[k], Ws_h_d[k])
            qfbT_sb = pack_sb[:, 0:128].bitcast(f32).rearrange(
                "p (c t) -> p c t", t=TC)
            vT_sb = pack_sb[:, 128:136]
            qT_sb = pack_sb[:, 136:200].rearrange("p (k t) -> p k t", t=TC)
            mask_sb = singles.tile([1, S], f32, tag="mask")
            nc.sync.dma_start(mask_sb, maskterm_d[:])
            bc_sb = singles.tile([1, H], f32r, tag="bc")
            nc.sync.dma_start(bc_sb, bc_d[:].bitcast(f32r))

            # ---- sfT = Ws^T @ statesT (kept fp32 for the tanh input),
            # interleaved with the first 3 t's of the tanh phase so the ACT
            # engine saturates as soon as each sfT chunk lands. ----
            align_sb = singles.tile([TC, S], f32r, tag="align")
            sfT_sb = [singles.tile([P, S], f32, tag=f"sfT{d}", name=f"sfT{d}")
                      for d in range(KD2)]

            def group_alloc(group):
                return {t: [ps_al.tile([1, 512], f32, tag="al",
                                       name=f"al{t}_{blk}") for blk in range(2)]
                        for t in group}

            def emit_group_chunk(group, als, c):
                """tanh + v-matmuls for the t's of `group` on sfT chunk c."""
                for t in group:
                    th = tanhp.tile([P, S], f32r, tag="tanh")
                    nc.scalar.activation(
                        th, sfT_sb[c], AF.Tanh,
                        bias=qfbT_sb[:, c, t:t + 1], scale=1.0,
                    )
                    for blk in range(2):
                        nc.tensor.matmul(
                            als[t][blk], vT_sb[:, c:c + 1],
                            th[:, blk * 512:(blk + 1) * 512],
                            start=(c == 0), stop=(c == KD2 - 1),
                        )

            last_stg = [None]

            def emit_group_stage(group, als):
                """mask-add + copy each alignment row out of PSUM, then
                expand to the (TC, S) align_sb layout via tiny DMAs. Row 0
                (processed last) is written directly: DVE can write
                partition 0, skipping the staging DMA on the critical
                path into the exp."""
                for t in group:
                    if t == 0:
                        for blk in range(2):
                            nc.vector.tensor_add(
                                align_sb[0:1, blk * 512:(blk + 1) * 512],
                                als[t][blk],
                                mask_sb[:, blk * 512:(blk + 1) * 512],
                            )
                        continue
                    stg = stagep.tile([1, S], f32r, tag="stg", name=f"stg{t}")
                    for blk in range(2):
                        nc.vector.tensor_add(
                            stg[0:1, blk * 512:(blk + 1) * 512],
                            als[t][blk],
                            mask_sb[:, blk * 512:(blk + 1) * 512],
                        )
                    nc.sync.dma_start(align_sb[t:t + 1, :], stg)
                    last_stg[0] = stg

            groups = [(1, 2, 3), (4, 5, 6), (7, 8, 9), (10, 11, 12),
                      (13, 14), (15,), (0,)]
            als0 = group_alloc(groups[0])
            for dout in range(KD2):
                for blk in range(2):
                    pm = ps_big.tile([P, 512], f32, tag="mm512")
                    for din in range(KD2):
                        nc.tensor.matmul(
                            pm,
                            Ws_sb[dout][:, din, :],
                            statesT_sb[din][:, blk * 512:(blk + 1) * 512],
                            start=(din == 0), stop=(din == KD2 - 1),
                        )
                    nc.vector.tensor_copy(
                        sfT_sb[dout][:, blk * 512:(blk + 1) * 512], pm)
                emit_group_chunk(groups[0], als0, dout)
            emit_group_stage(groups[0], als0)

            for g in groups[1:]:
                als = group_alloc(g)
                for c in range(KD2):
                    emit_group_chunk(g, als, c)
                emit_group_stage(g, als)

            # ---- f32r states + Wc: only needed by the context/hidden tail;
            # fetched in the background during the tanh phase ----
            states_sb = [singles.tile([P, D2], f32r, tag=f"st{sb}", name=f"st{sb}")
                         for sb in range(NB)]
            for sb in range(NB):
                nc.sync.dma_start(
                    states_sb[sb], states_d[sb * P:(sb + 1) * P, :].bitcast(f32r))
            Wc_sb = singles.tile([P, KD3, H], f32r, tag="Wc")
            nc.sync.dma_start(
                Wc_sb, Wc_d[:].rearrange("(k p) n -> p k n", p=P).bitcast(f32r))
            Wc1h_sb = singles.tile([P, KD2, H], f16, tag="Wc1h")
            nc.sync.dma_start(
                Wc1h_sb, Wc1_h_d[:].rearrange("(k p) n -> p k n", p=P))

            # ---- M1 = states @ Wc[:D2] (fp16 inputs, fp32 accumulate) on
            # the PE's spare capacity during the ACT-bound tanh phase.
            # attention_hidden = att @ M1 + q @ Wc[D2:] + bc  (reassociated)
            M1_sb = [singles.tile([P, H], f32r, tag=f"M1{sb}", name=f"M1{sb}")
                     for sb in range(NB)]
            for sb in range(NB):
                pm1 = ps_big.tile([P, H], f32, tag="mm512")
                for din in range(KD2):
                    nc.tensor.matmul(
                        pm1, statesT_sb[din][:, sb * P:(sb + 1) * P],
                        Wc1h_sb[:, din, :],
                        start=(din == 0), stop=(din == KD2 - 1),
                    )
                nc.vector.tensor_copy(M1_sb[sb], pm1)

            # ---- softmax over s (batched across the 16 t rows) ----
            # No max-subtraction: |align| <= sum(|v|) ~= 16.4, well inside
            # fp32 exp range, and masked entries are exp(-1e9) = 0.
            den = singles.tile([TC, 1], f32, tag="den")
            rden = singles.tile([TC, 1], f32, tag="rden")
            att_sb = singles.tile([TC, S], f32r, tag="att")
            nc.scalar.activation(
                att_sb, align_sb, AF.Exp, bias=0.0, scale=1.0, accum_out=den)
            nc.vector.reciprocal(rden, den)

            # ---- attT (also the attentions_t output) ----
            # Scale per s-chunk so each transpose starts right after its
            # chunk's scale instead of after the full-row scale.
            attT_sb = singles.tile([P, NB, TC], f32r, tag="attT")
            for sb in range(NB):
                nc.vector.tensor_scalar_mul(
                    att_sb[:, sb * P:(sb + 1) * P],
                    att_sb[:, sb * P:(sb + 1) * P], rden)
                pt = ps_al.tile([P, TC], f32r, tag="al", padded_shape=[P, 512])
                nc.tensor.transpose(
                    pt, att_sb[:, sb * P:(sb + 1) * P], ident[:TC, :TC])
                nc.vector.tensor_copy(attT_sb[:, sb, :], pt)
            nc.sync.dma_start(
                attT_d[:].rearrange("(c p) t -> p c t", p=P).bitcast(f32r), attT_sb)

            # ---- context (natural layout), one tile per half so half 0's
            # output DMA overlaps half 1's matmuls ----
            ctx_sb = [singles.tile([TC, 512], f32, tag=f"ctx{nh}", name=f"ctx{nh}")
                      for nh in range(2)]
            for nh in range(2):
                pc = ps_big.tile([TC, 512], f32, tag="mm512")
                for sb in range(NB):
                    nc.tensor.matmul(
                        pc, attT_sb[:, sb, :],
                        states_sb[sb][:, nh * 512:(nh + 1) * 512],
                        start=(sb == 0), stop=(sb == NB - 1),
                    )
                nc.vector.tensor_copy(ctx_sb[nh], pc)
                nc.sync.dma_start(ctx_d[:, nh * 512:(nh + 1) * 512], ctx_sb[nh])

            # ---- attention_hidden: the bias + query part accumulated
            # into PSUM ahead of time; the context part joins at the tail ----
            hid_sb = singles.tile([TC, H], f32, tag="hid")
            ph = ps_big.tile([TC, H], f32, tag="mm512")
            nc.tensor.matmul(ph, onesT, bc_sb, start=True, stop=False)
            for k in range(KH):
                nc.tensor.matmul(
                    ph, qT_sb[:, k, :], Wc_sb[:, KD2 + k, :],
                    start=False, stop=False,
                )
            for sb in range(NB):
                nc.tensor.matmul(
                    ph, attT_sb[:, sb, :], M1_sb[sb],
                    start=False, stop=(sb == NB - 1),
                )
            nc.vector.tensor_copy(hid_sb, ph)
            nc.sync.dma_start(hid_d[:], hid_sb)

            # Keep the PE's HAM clock warm across the exp/softmax window so
            # the context/hidden matmuls run at full rate. These dummy
            # matmuls only become ready once the last alignment row lands.
            warm = ps_al.tile([1, 512], f32, tag="al", padded_shape=[P, 512])
            for _ in range(20):
                nc.tensor.matmul(
                    warm, ones1, align_sb[0:1, 0:512],
                    start=True, stop=True, skip_group_check=True,
                )

    nc.compile()
    return nc


_NC = None


def _get_nc():
    global _NC
    if _NC is None:
        _NC = _build()
    return _NC


def kernel(query, states, source_mask, Wq, bq, Ws, v, Wc, bc):
    query = np.asarray(query, dtype=np.float32)
    states = np.asarray(states, dtype=np.float32)
    source_mask = np.asarray(source_mask)
    Wq = np.ascontiguousarray(np.asarray(Wq, dtype=np.float32))
    bq = np.ascontiguousarray(np.asarray(bq, dtype=np.float32))
    Ws_f16 = np.asarray(Ws, dtype=np.float32).astype(np.float16)
    Ws_h = np.ascontiguousarray(
        Ws_f16.reshape(KD2, P, KD2, P).transpose(2, 1, 0, 3))
    v = np.ascontiguousarray(np.asarray(v, dtype=np.float32))
    Wc = np.ascontiguousarray(np.asarray(Wc, dtype=np.float32))
    Wc1_h = np.ascontiguousarray(Wc[:D2].astype(np.float16))
    bc = np.ascontiguousarray(np.asarray(bc, dtype=np.float32)).reshape(1, H)

    nc = _get_nc()
    in_maps = []
    for c in range(NCORES):
        b, h = divmod(c, 2)
        tsl = slice(h * TC, (h + 1) * TC)
        maskterm = (source_mask[b].astype(np.float32) - 1.0) * 1e9
        qfbT = (query[b, tsl, :] @ Wq + bq).T.astype(np.float32)
        qT = np.ascontiguousarray(query[b, tsl, :].T)
        states_b = np.ascontiguousarray(states[b])
        pack = np.empty((P, 200), np.float32)
        pack[:, 0:128] = qfbT.reshape(KD2, P, TC).transpose(1, 0, 2).reshape(P, 128)
        pack[:, 128:136] = v.reshape(KD2, P).T
        pack[:, 136:200] = qT.reshape(KH, P, TC).transpose(1, 0, 2).reshape(P, 64)
        in_maps.append({
            "smallpack": pack,
            "states": states_b,
            "statesT_h": np.ascontiguousarray(states_b.astype(np.float16).T),
            "maskterm": np.ascontiguousarray(maskterm.reshape(1, S)),
            "Ws_h": Ws_h, "Wc": Wc, "Wc1_h": Wc1_h, "bc": bc,
        })

    res = run_bass_kernel_spmd(nc, in_maps, core_ids=list(range(NCORES)))

    ctx_out = np.empty((B, T, D2), np.float32)
    hid_out = np.empty((B, T, H), np.float32)
    attT_out = np.empty((B, S, T), np.float32)
    for c in range(NCORES):
        b, h = divmod(c, 2)
        tsl = slice(h * TC, (h + 1) * TC)
        r = res.results[c]
        ctx_out[b, tsl] = r["out_ctx"]
        hid_out[b, tsl] = r["out_hid"]
        attT_out[b, :, tsl] = r["out_attT"]
    return ctx_out, hid_out, attT_out
